# revision 3
# baseline (speedup 1.0000x reference)
"""Trainium2 Bass kernel for nn_HarmonicNoiseOscillator — v3.

out = tanh(vm^2*HS + S*noise*(0.333-0.133*vm)),  HS = sum_h w_h sin(2*pi*h*Z).

With theta' = 2*pi*Z - pi (mod 2pi) and w'_h = (-1)^h w_h:
  HS = s*P(c), s = sin theta', c = cos theta', P = sum w'_h U_{h-1}.
Half-angle: u = pi*Z - 5*pi/4 - pi*K(block)  (matmul-computed, in-table):
  s = 2*sh^2 - 1, c = 2*sh*ch,  sh = sin(u), ch = sin(u+pi/2).
Per-16-sample-block integer offsets K keep u and u+pi/2 within [-pi, pi]
(Sin's exact range; K shifts by pi flip sh,ch together and cancel in
sh*ch and sh^2).

P factored: P(c) = a7*(c-r1)*prod_i(c^2+p_i c+q_i); with c = 2cc:
  (cc - r1/2) * prod_i[(cc+p_i/4)^2 + (q_i/4 - p_i^2/16)] * 128*a7.

Engines: PE fp32 z-matmul (k=76) + fp16 vm matmul; ACT: sh, ch,
y=Square(sh), vmsq=Square(vm from PSUM), tanh (batched per function);
DVE: fp16 ts/tt chain; gpsimd: slice of the final add.
"""

import math
import os

import numpy as np

SR = 22050.0
FRAME = 256
NH = 8
N, L = 16, 512
T = L * FRAME
NCORES = 8
NPC = N // NCORES
P = 128
SEG = 4
FD = SEG * FRAME          # 1024 per batch row
CFD = NPC * FD            # 2048 free per core (both rows side by side)
NCH = 2                   # chunks for pipelining
CW = CFD // NCH           # 1024
BLOCK = 16
NBLK = FRAME // BLOCK
KZ = SEG * (3 + NBLK)     # 76
GP_W = 512                # gpsimd slice of the final add

_NC_CACHE = {}
LAST_RESULTS = None


def _ensure_ntff_hook():
    try:
        import sys
        import types

        try:
            import antenv.axon_hooks  # noqa: F401
            return
        except ImportError:
            pass
        mod = types.ModuleType("antenv.axon_hooks")
        mod._hook = None

        def set_axon_ntff_profile_hook(hook, _m=mod):
            _m._hook = hook

        def get_axon_ntff_profile_hook(_m=mod):
            return _m._hook

        mod.set_axon_ntff_profile_hook = set_axon_ntff_profile_hook
        mod.get_axon_ntff_profile_hook = get_axon_ntff_profile_hook
        sys.modules["antenv.axon_hooks"] = mod
        try:
            import antenv

            antenv.axon_hooks = mod
        except ImportError:
            pass
        try:
            from trn_agent_boot.trn_boot import _ntff_profile_via_ctypes

            so_path = "/opt/axon/libaxon_pjrt.so"
            if os.path.exists(so_path):
                hook = _ntff_profile_via_ctypes(so_path)
                if hook is not None:
                    mod._hook = hook
        except Exception:
            pass
    except Exception:
        pass


_ensure_ntff_hook()


def _interp_consts():
    s = np.arange(FRAME, dtype=np.float64)
    w1 = 0.5 + (s + 0.5) / 256.0
    w2 = (s + 0.5) / 256.0 - 0.5
    c1 = np.where(s < 128, 1.0 - w1, 0.0)
    c2 = np.where(s < 128, w1, 1.0 - w2)
    c3 = np.where(s < 128, 0.0, w2)
    return c1, c2, c3, np.cumsum(c1), np.cumsum(c2), np.cumsum(c3)


def _neighbors(x):
    prev = np.concatenate([x[:, :1], x[:, :-1]], axis=1)
    nxt = np.concatenate([x[:, 1:], x[:, -1:]], axis=1)
    return prev, x, nxt


def _poly_factors(w):
    from numpy.polynomial import polynomial as npoly

    U = [np.array([1.0]), np.array([0.0, 2.0])]
    for _ in range(2, NH):
        U.append(npoly.polysub(npoly.polymul(np.array([0.0, 2.0]), U[-1]), U[-2]))
    coef = np.zeros(NH)
    for h in range(1, NH + 1):
        coef[:h] += ((-1) ** h) * w[h - 1] * U[h - 1]
    a7 = coef[-1]
    roots = np.roots(coef[::-1])
    real = sorted(r.real for r in roots if abs(r.imag) < 1e-9)
    cplx = [r for r in roots if r.imag > 1e-9]
    quads = [(-2 * r.real, (r * r.conjugate()).real) for r in cplx]
    assert len(real) % 2 == 1
    r1 = real.pop(len(real) // 2)
    while real:
        a, b = real.pop(0), real.pop(-1)
        quads.append((-(a + b), a * b))
    assert len(quads) == 3
    return float(a7), float(r1), [(float(p), float(q)) for p, q in quads]


def _build_nc(sc):
    import concourse.bacc as bacc
    import concourse.mybir as mybir
    import concourse.tile as tile
    import concourse.bass as bass

    f32 = mybir.dt.float32
    fp16 = mybir.dt.float16
    Act = mybir.ActivationFunctionType
    Alu = mybir.AluOpType
    PI = math.pi

    nc = bacc.Bacc(
        "TRN2", target_bir_lowering=False, debug=False, num_devices=NCORES
    )

    # noise/out are [P, CFD]: col = n*FD + f (host packs/unpacks)
    # zblob: [KZ, FD + NPC*P] f32 = rhs_z || lhs_z[0] || lhs_z[1]
    # vblob: [12, FD + NPC*P] fp16 = rhs_vm || lhs_vm[0] || lhs_vm[1]
    noise_d = nc.dram_tensor("noise", [P, CFD], fp16, kind="ExternalInput")
    zblob_d = nc.dram_tensor("zblob", [KZ, FD + NPC * P], f32,
                             kind="ExternalInput")
    vblob_d = nc.dram_tensor("vblob", [12, FD + NPC * P], fp16,
                             kind="ExternalInput")
    out_d = nc.dram_tensor("out", [P, CFD], fp16, kind="ExternalOutput")

    (p2, q2), (p3, q3), (p4, q4) = sc["quads"]
    r1 = sc["r1"]
    kappa = sc["kappa"]       # 128*a7/g
    Rc = sc["R"]
    g = sc["g"]
    pp = [p2 / 4.0, p3 / 4.0, p4 / 4.0]
    dd = [q2 / 4.0 - p2 * p2 / 16.0, q3 / 4.0 - p3 * p3 / 16.0,
          q4 / 4.0 - p4 * p4 / 16.0]

    MM = 512  # matmul column width (one PSUM bank worth of fp32)

    with tile.TileContext(nc) as tc:
        with (
            tc.tile_pool(name="const", bufs=1) as cpool,
            tc.tile_pool(name="work", bufs=NCH) as pool,
            tc.tile_pool(name="big", bufs=1) as bigpool,
            tc.tile_pool(name="psum", bufs=1, space=bass.MemorySpace.PSUM) as psum,
        ):
            # zblob layout: [lhs0 | lhs1 | rhs cols 0..FD); first transfer
            # covers lhs + first rhs half so matmul 0 starts early
            ZB0 = NPC * P + 512
            zblob_t = cpool.tile([KZ, FD + NPC * P], f32)
            nc.sync.dma_start(zblob_t[:, 0:ZB0], zblob_d[:, 0:ZB0])
            nc.sync.dma_start(zblob_t[:, ZB0:], zblob_d[:, ZB0:])
            vblob_t = cpool.tile([12, FD + NPC * P], fp16)
            nc.scalar.dma_start(vblob_t[:], vblob_d[:])
            noise_t = bigpool.tile([P, CFD], fp16)
            nc.gpsimd.dma_start(noise_t[:], noise_d[:])
            # act bias constants: col 0 = pi/2 (ch), col 1 = R (vm^2 recentre)
            cb_t = cpool.tile([P, 4], f32)
            nc.vector.memset(cb_t[:, 0:1], PI / 2.0)
            nc.vector.memset(cb_t[:, 1:2], Rc)
            nc.vector.memset(cb_t[:, 2:3], pp[0])
            nc.vector.memset(cb_t[:, 3:4], pp[1])
            _ = Alu  # keep import used even if ops below change

            # per-row PSUM tiles: dependency granularity = one batch row,
            # so chunk 0's sins start as soon as row 0's matmuls land
            zps = [psum.tile([P, FD], f32, tag=f"zp{n}", name=f"zp{n}")
                   for n in range(NPC)]
            vps = [psum.tile([P, FD], f32, tag=f"vp{n}", name=f"vp{n}")
                   for n in range(NPC)]

            rhsz_t = zblob_t[:, NPC * P : NPC * P + FD]
            rhsv_t = vblob_t[:, 0:FD]
            lhsz_ts = [zblob_t[:, n * P : (n + 1) * P] for n in range(NPC)]
            lhsv_ts = [vblob_t[:, FD + n * P : FD + (n + 1) * P]
                       for n in range(NPC)]

            # matmuls: z then vm per batch row, so chunk 0's consumers
            # (sin/vfac/square) can start while row 1 is still on the PE
            for n in range(NPC):
                for m in range(FD // MM):
                    rcols = bass.ts(m, MM)
                    nc.tensor.matmul(zps[n][:, rcols], lhsz_ts[n],
                                     rhsz_t[:, rcols])
                for m in range(FD // MM):
                    rcols = bass.ts(m, MM)
                    nc.tensor.matmul(vps[n][:, rcols], lhsv_ts[n],
                                     rhsv_t[:, rcols])

            # ---- phase 1: Sin + Square passes (act; tables coexist) ----
            # vp holds (vm - R); Square with bias R recovers vm^2.
            sh, chh, ysq, vmsq = [], [], [], []
            for ci in range(NCH):
                t = pool.tile([P, CW], fp16, tag="sh")
                nc.scalar.activation(t[:], zps[ci][:], Act.Sin)
                sh.append(t)
                t = pool.tile([P, CW], fp16, tag="chh")
                nc.scalar.activation(t[:], zps[ci][:], Act.Sin,
                                     bias=cb_t[:, 0:1])
                chh.append(t)
                t = pool.tile([P, CW], fp16, tag="ysq")
                nc.scalar.activation(t[:], sh[ci][:], Act.Square)
                ysq.append(t)
                t = pool.tile([P, CW], fp16, tag="vmsq")
                nc.scalar.activation(t[:], vps[ci][:], Act.Square,
                                     bias=cb_t[:, 1:2])
                vmsq.append(t)

            # ---- phase 2: DVE chain per chunk ----
            pre = []
            for ci in range(NCH):
                cs = bass.ts(ci, CW)
                nz = pool.tile([P, CW], fp16, tag="nz")
                nc.vector.tensor_mul(nz[:], vps[ci][:], noise_t[:, cs])
                cc = pool.tile([P, CW], fp16, tag="cc")
                nc.vector.tensor_mul(cc[:], sh[ci][:], chh[ci][:])
                sfac = pool.tile([P, CW], fp16, tag="sfac")
                nc.vector.tensor_scalar(sfac[:], ysq[ci][:], 2.0, -1.0,
                                        Alu.mult, Alu.add)
                lin2 = pool.tile([P, CW], fp16, tag="lin2")
                nc.vector.tensor_scalar(lin2[:], cc[:], -r1 / 2.0, kappa,
                                        Alu.add, Alu.mult)
                qt = []
                # quads 0/1: squares on the act engine (tables coexist)
                for qi in range(2):
                    sqa = pool.tile([P, CW], fp16, tag=f"sqa{qi}",
                                    name=f"sqa{qi}")
                    nc.scalar.activation(sqa[:], cc[:], Act.Square,
                                         bias=cb_t[:, 2 + qi : 3 + qi])
                    q = pool.tile([P, CW], fp16, tag=f"q{qi}", name=f"q{qi}")
                    nc.vector.tensor_scalar(q[:], sqa[:], dd[qi], None, Alu.add)
                    qt.append(q)
                for qi in range(2, 3):
                    ccp = pool.tile([P, CW], fp16, tag=f"ccp{qi}")
                    nc.vector.tensor_scalar(ccp[:], cc[:], pp[qi], None, Alu.add)
                    sq = pool.tile([P, CW], fp16, tag=f"sq{qi}")
                    nc.vector.tensor_mul(sq[:], ccp[:], ccp[:])
                    q = pool.tile([P, CW], fp16, tag=f"q{qi}", name=f"qq{qi}")
                    nc.vector.tensor_scalar(q[:], sq[:], dd[qi], None, Alu.add)
                    qt.append(q)
                p1 = pool.tile([P, CW], fp16, tag="p1")
                nc.vector.tensor_mul(p1[:], qt[0][:], qt[1][:])
                p2t = pool.tile([P, CW], fp16, tag="p2")
                nc.vector.tensor_mul(p2t[:], p1[:], qt[2][:])
                p3t = pool.tile([P, CW], fp16, tag="p3")
                nc.vector.tensor_mul(p3t[:], p2t[:], lin2[:])
                p4t = pool.tile([P, CW], fp16, tag="p4")
                nc.vector.tensor_mul(p4t[:], p3t[:], vmsq[ci][:])
                At = pool.tile([P, CW], fp16, tag="A")
                nc.vector.tensor_mul(At[:], p4t[:], sfac[:])
                prt = pool.tile([P, CW], fp16, tag="pre")
                nc.vector.tensor_add(prt[:], At[:], nz[:])
                pre.append(prt)

            # ---- phase 3: tanh + out DMA ----
            for ci in range(NCH):
                cs = bass.ts(ci, CW)
                o = pool.tile([P, CW], fp16, tag="o")
                nc.scalar.activation(o[:], pre[ci][:], Act.Tanh, scale=g)
                nc.sync.dma_start(out_d[:, cs], o[:])

    nc.compile()
    return nc


def _host_inputs(f0, uv, weight, noise):
    f0 = np.asarray(f0, np.float64).reshape(N, L)
    uv = np.asarray(uv, np.float64).reshape(N, L)
    weight = np.asarray(weight, np.float64).reshape(NH)
    noise_h = np.ascontiguousarray(
        np.asarray(noise, np.float32).reshape(N, T)
    ).astype(np.float16)

    c1, c2, c3, A, B, C = _interp_consts()
    Fp, Fc, Fn = _neighbors(f0)
    Up, Uc, Un = _neighbors(uv)

    FS = Fp * A[-1] + Fc * B[-1] + Fn * C[-1]
    C0 = np.concatenate([np.zeros((N, 1)), np.cumsum(FS, axis=1)[:, :-1]], axis=1)
    D1 = np.mod(C0 / SR, 1.0)

    w = np.exp(weight)
    w = w / max(np.sqrt((w * w).sum()), 1e-12)
    S = float(w.sum())
    a7, r1, quads = _poly_factors(w)
    g = -0.133 * S
    Rc = 0.333 / 0.133
    sc = {"kappa": 128.0 * a7 / g, "R": Rc, "g": g,
          "r1": r1, "quads": quads}

    PI = math.pi
    # rhs_z: A/B/C prefix rows + per-block indicator rows, per segment
    rhs_z = np.zeros((KZ, FD), np.float64)
    for seg in range(SEG):
        base = seg * (3 + NBLK)
        cb = seg * FRAME
        rhs_z[base + 0, cb : cb + FRAME] = A
        rhs_z[base + 1, cb : cb + FRAME] = B
        rhs_z[base + 2, cb : cb + FRAME] = C
        for blk in range(NBLK):
            rhs_z[base + 3 + blk, cb + blk * BLOCK : cb + (blk + 1) * BLOCK] = 1.0
    rhs_z = rhs_z.astype(np.float32)

    mididx = np.arange(NBLK) * BLOCK + BLOCK // 2
    Amid, Bmid, Cmid = A[mididx], B[mididx], C[mididx]

    jidx = 4 * np.arange(P)[None, :] + np.arange(SEG)[:, None]

    rhs_v = np.zeros((12, FD), np.float64)
    for seg in range(SEG):
        cb = seg * FRAME
        rhs_v[seg * 3 + 0, cb : cb + FRAME] = c1
        rhs_v[seg * 3 + 1, cb : cb + FRAME] = c2
        rhs_v[seg * 3 + 2, cb : cb + FRAME] = c3
    rhs_v = rhs_v.astype(np.float16)

    in_maps = []
    for core in range(NCORES):
        rows = range(core * NPC, (core + 1) * NPC)
        lhs_z = np.zeros((NPC, KZ, P), np.float64)
        lhs_v = np.zeros((NPC, 12, P), np.float64)
        for ln, nr in enumerate(rows):
            for seg in range(SEG):
                j = jidx[seg]
                base = seg * (3 + NBLK)
                lhs_z[ln, base + 0] = PI * Fp[nr, j] / SR
                lhs_z[ln, base + 1] = PI * Fc[nr, j] / SR
                lhs_z[ln, base + 2] = PI * Fn[nr, j] / SR
                Zmid = (
                    D1[nr, j][None, :]
                    + (Fp[nr, j][None, :] * Amid[:, None]
                       + Fc[nr, j][None, :] * Bmid[:, None]
                       + Fn[nr, j][None, :] * Cmid[:, None]) / SR
                )  # [NBLK, P]
                K = np.round(Zmid) - 1.0
                for blk in range(NBLK):
                    lhs_z[ln, base + 3 + blk] = (
                        PI * D1[nr, j] - 5.0 * PI / 4.0 - PI * K[blk]
                    )
                # each row shifted by -R: c1+c2+c3 == 1 -> vp = vm - R
                lhs_v[ln, seg * 3 + 0] = Up[nr, j] - Rc
                lhs_v[ln, seg * 3 + 1] = Uc[nr, j] - Rc
                lhs_v[ln, seg * 3 + 2] = Un[nr, j] - Rc
        nco = noise_h[core * NPC : (core + 1) * NPC]  # [NPC, T]
        nmat = np.zeros((P, CFD), np.float16)
        for ln in range(NPC):
            nmat[:, ln * FD : (ln + 1) * FD] = nco[ln].reshape(P, FD)
        zblob = np.zeros((KZ, FD + NPC * P), np.float32)
        for ln in range(NPC):
            zblob[:, ln * P : (ln + 1) * P] = lhs_z[ln].astype(np.float32)
        zblob[:, NPC * P :] = rhs_z
        vblob = np.zeros((12, FD + NPC * P), np.float16)
        vblob[:, :FD] = rhs_v
        for ln in range(NPC):
            vblob[:, FD + ln * P : FD + (ln + 1) * P] = lhs_v[ln].astype(
                np.float16
            )
        in_maps.append({"noise": nmat, "zblob": zblob, "vblob": vblob})
    return in_maps, sc


def kernel(f0, uv, weight, noise):
    global LAST_RESULTS
    from concourse.bass_utils import run_bass_kernel_spmd

    in_maps, sc = _host_inputs(f0, uv, weight, noise)
    key = tuple(np.asarray(weight, np.float64).reshape(-1).tolist())
    if key not in _NC_CACHE:
        _NC_CACHE[key] = _build_nc(sc)
    nc = _NC_CACHE[key]
    res = run_bass_kernel_spmd(nc, in_maps, list(range(NCORES)))
    LAST_RESULTS = res
    out = np.empty((N, 1, T), dtype=np.float32)
    for core in range(NCORES):
        om = res.results[core]["out"].astype(np.float32)  # [P, CFD]
        for ln in range(NPC):
            out[core * NPC + ln, 0, :] = om[:, ln * FD : (ln + 1) * FD].reshape(T)
    return out


# revision 4
# speedup vs baseline: 1.0097x; 1.0097x over previous
"""Trainium2 Bass kernel for nn_HarmonicNoiseOscillator — v3.

out = tanh(vm^2*HS + S*noise*(0.333-0.133*vm)),  HS = sum_h w_h sin(2*pi*h*Z).

With theta' = 2*pi*Z - pi (mod 2pi) and w'_h = (-1)^h w_h:
  HS = s*P(c), s = sin theta', c = cos theta', P = sum w'_h U_{h-1}.
Half-angle: u = pi*Z - 5*pi/4 - pi*K(block)  (matmul-computed, in-table):
  s = 2*sh^2 - 1, c = 2*sh*ch,  sh = sin(u), ch = sin(u+pi/2).
Per-16-sample-block integer offsets K keep u and u+pi/2 within [-pi, pi]
(Sin's exact range; K shifts by pi flip sh,ch together and cancel in
sh*ch and sh^2).

P factored: P(c) = a7*(c-r1)*prod_i(c^2+p_i c+q_i); with c = 2cc:
  (cc - r1/2) * prod_i[(cc+p_i/4)^2 + (q_i/4 - p_i^2/16)] * 128*a7.

Engines: PE fp32 z-matmul (k=76) + fp16 vm matmul; ACT: sh, ch,
y=Square(sh), vmsq=Square(vm from PSUM), tanh (batched per function);
DVE: fp16 ts/tt chain; gpsimd: slice of the final add.
"""

import math
import os

import numpy as np

SR = 22050.0
FRAME = 256
NH = 8
N, L = 16, 512
T = L * FRAME
NCORES = 8
NPC = N // NCORES
P = 128
SEG = 4
FD = SEG * FRAME          # 1024 per batch row
CFD = NPC * FD            # 2048 free per core (both rows side by side)
NCH = 2                   # chunks for pipelining
CW = CFD // NCH           # 1024
BLOCK = 32
NBLK = FRAME // BLOCK     # 8
# bf16 hi/lo split rows per segment: 3x3 ABC (LhRh, LhRl, LlRh) + 2x
# NBLK block-offset rows (Vh, Vl)
KSEG = 9 + 2 * NBLK       # 25
KZ = SEG * KSEG           # 100
GP_W = 512                # gpsimd slice of the final add

_NC_CACHE = {}
LAST_RESULTS = None


def _ensure_ntff_hook():
    try:
        import sys
        import types

        try:
            import antenv.axon_hooks  # noqa: F401
            return
        except ImportError:
            pass
        mod = types.ModuleType("antenv.axon_hooks")
        mod._hook = None

        def set_axon_ntff_profile_hook(hook, _m=mod):
            _m._hook = hook

        def get_axon_ntff_profile_hook(_m=mod):
            return _m._hook

        mod.set_axon_ntff_profile_hook = set_axon_ntff_profile_hook
        mod.get_axon_ntff_profile_hook = get_axon_ntff_profile_hook
        sys.modules["antenv.axon_hooks"] = mod
        try:
            import antenv

            antenv.axon_hooks = mod
        except ImportError:
            pass
        try:
            from trn_agent_boot.trn_boot import _ntff_profile_via_ctypes

            so_path = "/opt/axon/libaxon_pjrt.so"
            if os.path.exists(so_path):
                hook = _ntff_profile_via_ctypes(so_path)
                if hook is not None:
                    mod._hook = hook
        except Exception:
            pass
    except Exception:
        pass


_ensure_ntff_hook()


def _interp_consts():
    s = np.arange(FRAME, dtype=np.float64)
    w1 = 0.5 + (s + 0.5) / 256.0
    w2 = (s + 0.5) / 256.0 - 0.5
    c1 = np.where(s < 128, 1.0 - w1, 0.0)
    c2 = np.where(s < 128, w1, 1.0 - w2)
    c3 = np.where(s < 128, 0.0, w2)
    return c1, c2, c3, np.cumsum(c1), np.cumsum(c2), np.cumsum(c3)


def _neighbors(x):
    prev = np.concatenate([x[:, :1], x[:, :-1]], axis=1)
    nxt = np.concatenate([x[:, 1:], x[:, -1:]], axis=1)
    return prev, x, nxt


def _poly_factors(w):
    from numpy.polynomial import polynomial as npoly

    U = [np.array([1.0]), np.array([0.0, 2.0])]
    for _ in range(2, NH):
        U.append(npoly.polysub(npoly.polymul(np.array([0.0, 2.0]), U[-1]), U[-2]))
    coef = np.zeros(NH)
    for h in range(1, NH + 1):
        coef[:h] += ((-1) ** h) * w[h - 1] * U[h - 1]
    a7 = coef[-1]
    roots = np.roots(coef[::-1])
    real = sorted(r.real for r in roots if abs(r.imag) < 1e-9)
    cplx = [r for r in roots if r.imag > 1e-9]
    quads = [(-2 * r.real, (r * r.conjugate()).real) for r in cplx]
    assert len(real) % 2 == 1
    r1 = real.pop(len(real) // 2)
    while real:
        a, b = real.pop(0), real.pop(-1)
        quads.append((-(a + b), a * b))
    assert len(quads) == 3
    return float(a7), float(r1), [(float(p), float(q)) for p, q in quads]


def _build_nc(sc):
    import concourse.bacc as bacc
    import concourse.mybir as mybir
    import concourse.tile as tile
    import concourse.bass as bass

    f32 = mybir.dt.float32
    fp16 = mybir.dt.float16
    Act = mybir.ActivationFunctionType
    Alu = mybir.AluOpType
    PI = math.pi

    nc = bacc.Bacc(
        "TRN2", target_bir_lowering=False, debug=False, num_devices=NCORES
    )

    bf16 = mybir.dt.bfloat16
    # noise/out are [P, CFD]: col = n*FD + f (host packs/unpacks)
    # zblob: [KZ, NPC*P + FD] bf16 = lhs_z[0] || lhs_z[1] || rhs_z
    # vblob: [12, FD + NPC*P] fp16 = rhs_vm || lhs_vm[0] || lhs_vm[1]
    noise_d = nc.dram_tensor("noise", [P, CFD], fp16, kind="ExternalInput")
    zblob_d = nc.dram_tensor("zblob", [KZ, FD + NPC * P], bf16,
                             kind="ExternalInput")
    vblob_d = nc.dram_tensor("vblob", [12, FD + NPC * P], fp16,
                             kind="ExternalInput")
    out_d = nc.dram_tensor("out", [P, CFD], fp16, kind="ExternalOutput")

    (p2, q2), (p3, q3), (p4, q4) = sc["quads"]
    r1 = sc["r1"]
    kappa = sc["kappa"]       # 128*a7/g
    Rc = sc["R"]
    g = sc["g"]
    pp = [p2 / 4.0, p3 / 4.0, p4 / 4.0]
    dd = [q2 / 4.0 - p2 * p2 / 16.0, q3 / 4.0 - p3 * p3 / 16.0,
          q4 / 4.0 - p4 * p4 / 16.0]

    MM = 512  # matmul column width (one PSUM bank worth of fp32)

    with tile.TileContext(nc) as tc:
        with (
            tc.tile_pool(name="const", bufs=1) as cpool,
            tc.tile_pool(name="work", bufs=NCH) as pool,
            tc.tile_pool(name="big", bufs=1) as bigpool,
            tc.tile_pool(name="psum", bufs=1, space=bass.MemorySpace.PSUM) as psum,
        ):
            # zblob layout: [lhs0 | lhs1 | rhs cols 0..FD); first transfer
            # covers lhs + first rhs half so matmul 0 starts early
            ZB0 = NPC * P + 512
            zblob_t = cpool.tile([KZ, FD + NPC * P], bf16)
            nc.sync.dma_start(zblob_t[:, 0:ZB0], zblob_d[:, 0:ZB0])
            nc.sync.dma_start(zblob_t[:, ZB0:], zblob_d[:, ZB0:])
            vblob_t = cpool.tile([12, FD + NPC * P], fp16)
            nc.scalar.dma_start(vblob_t[:], vblob_d[:])
            noise_t = bigpool.tile([P, CFD], fp16)
            nc.gpsimd.dma_start(noise_t[:], noise_d[:])
            # act bias constants: col 0 = pi/2 (ch), col 1 = R (vm^2 recentre)
            cb_t = cpool.tile([P, 4], f32)
            nc.vector.memset(cb_t[:, 0:1], PI / 2.0)
            nc.vector.memset(cb_t[:, 1:2], Rc)
            nc.vector.memset(cb_t[:, 2:3], pp[0])
            nc.vector.memset(cb_t[:, 3:4], pp[1])
            _ = Alu  # keep import used even if ops below change

            # per-row PSUM tiles: dependency granularity = one batch row,
            # so chunk 0's sins start as soon as row 0's matmuls land
            zps = [psum.tile([P, FD], f32, tag=f"zp{n}", name=f"zp{n}")
                   for n in range(NPC)]
            vps = [psum.tile([P, FD], f32, tag=f"vp{n}", name=f"vp{n}")
                   for n in range(NPC)]

            rhsz_t = zblob_t[:, NPC * P : NPC * P + FD]
            rhsv_t = vblob_t[:, 0:FD]
            lhsz_ts = [zblob_t[:, n * P : (n + 1) * P] for n in range(NPC)]
            lhsv_ts = [vblob_t[:, FD + n * P : FD + (n + 1) * P]
                       for n in range(NPC)]

            # matmuls: z then vm per batch row, so chunk 0's consumers
            # (sin/vfac/square) can start while row 1 is still on the PE
            for n in range(NPC):
                for m in range(FD // MM):
                    rcols = bass.ts(m, MM)
                    nc.tensor.matmul(zps[n][:, rcols], lhsz_ts[n],
                                     rhsz_t[:, rcols])
                for m in range(FD // MM):
                    rcols = bass.ts(m, MM)
                    nc.tensor.matmul(vps[n][:, rcols], lhsv_ts[n],
                                     rhsv_t[:, rcols])

            # ---- phase 1: Sin + Square passes (act; tables coexist) ----
            # vp holds (vm - R); Square with bias R recovers vm^2.
            sh, chh, ysq, vmsq = [], [], [], []
            for ci in range(NCH):
                t = pool.tile([P, CW], fp16, tag="sh")
                nc.scalar.activation(t[:], zps[ci][:], Act.Sin)
                sh.append(t)
                t = pool.tile([P, CW], fp16, tag="chh")
                nc.scalar.activation(t[:], zps[ci][:], Act.Sin,
                                     bias=cb_t[:, 0:1])
                chh.append(t)
                t = pool.tile([P, CW], fp16, tag="ysq")
                nc.scalar.activation(t[:], sh[ci][:], Act.Square)
                ysq.append(t)
                t = pool.tile([P, CW], fp16, tag="vmsq")
                nc.scalar.activation(t[:], vps[ci][:], Act.Square,
                                     bias=cb_t[:, 1:2])
                vmsq.append(t)

            # ---- phase 2: DVE chain per chunk ----
            pre = []
            for ci in range(NCH):
                cs = bass.ts(ci, CW)
                nz = pool.tile([P, CW], fp16, tag="nz")
                nc.vector.tensor_mul(nz[:], vps[ci][:], noise_t[:, cs])
                cc = pool.tile([P, CW], fp16, tag="cc")
                nc.vector.tensor_mul(cc[:], sh[ci][:], chh[ci][:])
                sfac = pool.tile([P, CW], fp16, tag="sfac")
                nc.vector.tensor_scalar(sfac[:], ysq[ci][:], 2.0, -1.0,
                                        Alu.mult, Alu.add)
                lin2 = pool.tile([P, CW], fp16, tag="lin2")
                nc.vector.tensor_scalar(lin2[:], cc[:], -r1 / 2.0, kappa,
                                        Alu.add, Alu.mult)
                qt = []
                # quads 0/1: squares on the act engine (tables coexist)
                for qi in range(2):
                    sqa = pool.tile([P, CW], fp16, tag=f"sqa{qi}",
                                    name=f"sqa{qi}")
                    nc.scalar.activation(sqa[:], cc[:], Act.Square,
                                         bias=cb_t[:, 2 + qi : 3 + qi])
                    q = pool.tile([P, CW], fp16, tag=f"q{qi}", name=f"q{qi}")
                    nc.vector.tensor_scalar(q[:], sqa[:], dd[qi], None, Alu.add)
                    qt.append(q)
                for qi in range(2, 3):
                    ccp = pool.tile([P, CW], fp16, tag=f"ccp{qi}")
                    nc.vector.tensor_scalar(ccp[:], cc[:], pp[qi], None, Alu.add)
                    sq = pool.tile([P, CW], fp16, tag=f"sq{qi}")
                    nc.vector.tensor_mul(sq[:], ccp[:], ccp[:])
                    q = pool.tile([P, CW], fp16, tag=f"q{qi}", name=f"qq{qi}")
                    nc.vector.tensor_scalar(q[:], sq[:], dd[qi], None, Alu.add)
                    qt.append(q)
                p1 = pool.tile([P, CW], fp16, tag="p1")
                nc.vector.tensor_mul(p1[:], qt[0][:], qt[1][:])
                p2t = pool.tile([P, CW], fp16, tag="p2")
                nc.vector.tensor_mul(p2t[:], p1[:], qt[2][:])
                p3t = pool.tile([P, CW], fp16, tag="p3")
                nc.vector.tensor_mul(p3t[:], p2t[:], lin2[:])
                p4t = pool.tile([P, CW], fp16, tag="p4")
                nc.vector.tensor_mul(p4t[:], p3t[:], vmsq[ci][:])
                At = pool.tile([P, CW], fp16, tag="A")
                nc.vector.tensor_mul(At[:], p4t[:], sfac[:])
                prt = pool.tile([P, CW], fp16, tag="pre")
                nc.vector.tensor_add(prt[:], At[:], nz[:])
                pre.append(prt)

            # ---- phase 3: tanh + out DMA ----
            for ci in range(NCH):
                cs = bass.ts(ci, CW)
                o = pool.tile([P, CW], fp16, tag="o")
                nc.scalar.activation(o[:], pre[ci][:], Act.Tanh, scale=g)
                nc.sync.dma_start(out_d[:, cs], o[:])

    nc.compile()
    return nc


def _host_inputs(f0, uv, weight, noise):
    f0 = np.asarray(f0, np.float64).reshape(N, L)
    uv = np.asarray(uv, np.float64).reshape(N, L)
    weight = np.asarray(weight, np.float64).reshape(NH)
    noise_h = np.ascontiguousarray(
        np.asarray(noise, np.float32).reshape(N, T)
    ).astype(np.float16)

    c1, c2, c3, A, B, C = _interp_consts()
    Fp, Fc, Fn = _neighbors(f0)
    Up, Uc, Un = _neighbors(uv)

    FS = Fp * A[-1] + Fc * B[-1] + Fn * C[-1]
    C0 = np.concatenate([np.zeros((N, 1)), np.cumsum(FS, axis=1)[:, :-1]], axis=1)
    D1 = np.mod(C0 / SR, 1.0)

    w = np.exp(weight)
    w = w / max(np.sqrt((w * w).sum()), 1e-12)
    S = float(w.sum())
    a7, r1, quads = _poly_factors(w)
    g = -0.133 * S
    Rc = 0.333 / 0.133
    sc = {"kappa": 128.0 * a7 / g, "R": Rc, "g": g,
          "r1": r1, "quads": quads}

    PI = math.pi
    import ml_dtypes

    def bfh(x):
        return np.asarray(x, np.float64).astype(ml_dtypes.bfloat16).astype(
            np.float64
        )

    # rhs_z (bf16 split): per seg rows 0-2 ABC-hi, 3-5 ABC-lo (pairs with
    # lhs-hi), 6-8 ABC-hi (pairs with lhs-lo), 9.. block indicators x2
    ABC = np.stack([A, B, C])          # [3, FRAME] f64
    ABC_h = bfh(ABC)
    ABC_l = ABC - ABC_h                # exactly representable next chunk
    rhs_z = np.zeros((KZ, FD), np.float64)
    for seg in range(SEG):
        base = seg * KSEG
        cb = seg * FRAME
        rhs_z[base + 0 : base + 3, cb : cb + FRAME] = ABC_h
        rhs_z[base + 3 : base + 6, cb : cb + FRAME] = ABC_l
        rhs_z[base + 6 : base + 9, cb : cb + FRAME] = ABC_h
        for blk in range(NBLK):
            rhs_z[base + 9 + blk, cb + blk * BLOCK : cb + (blk + 1) * BLOCK] = 1.0
            rhs_z[base + 9 + NBLK + blk,
                  cb + blk * BLOCK : cb + (blk + 1) * BLOCK] = 1.0
    rhs_z = rhs_z.astype(ml_dtypes.bfloat16)

    mididx = np.arange(NBLK) * BLOCK + BLOCK // 2
    Amid, Bmid, Cmid = A[mididx], B[mididx], C[mididx]

    jidx = 4 * np.arange(P)[None, :] + np.arange(SEG)[:, None]

    rhs_v = np.zeros((12, FD), np.float64)
    for seg in range(SEG):
        cb = seg * FRAME
        rhs_v[seg * 3 + 0, cb : cb + FRAME] = c1
        rhs_v[seg * 3 + 1, cb : cb + FRAME] = c2
        rhs_v[seg * 3 + 2, cb : cb + FRAME] = c3
    rhs_v = rhs_v.astype(np.float16)

    in_maps = []
    for core in range(NCORES):
        rows = range(core * NPC, (core + 1) * NPC)
        lhs_z = np.zeros((NPC, KZ, P), np.float64)
        lhs_v = np.zeros((NPC, 12, P), np.float64)
        for ln, nr in enumerate(rows):
            for seg in range(SEG):
                j = jidx[seg]
                base = seg * KSEG
                Labc = np.stack([PI * Fp[nr, j] / SR, PI * Fc[nr, j] / SR,
                                 PI * Fn[nr, j] / SR])  # [3, P]
                # hi/lo in bf16: hi = bf16(L); lo = L - hi
                import ml_dtypes as _md

                Lh = Labc.astype(_md.bfloat16).astype(np.float64)
                Ll = Labc - Lh
                lhs_z[ln, base + 0 : base + 3] = Lh
                lhs_z[ln, base + 3 : base + 6] = Lh
                lhs_z[ln, base + 6 : base + 9] = Ll
                Zmid = (
                    D1[nr, j][None, :]
                    + (Fp[nr, j][None, :] * Amid[:, None]
                       + Fc[nr, j][None, :] * Bmid[:, None]
                       + Fn[nr, j][None, :] * Cmid[:, None]) / SR
                )  # [NBLK, P]
                K = np.round(Zmid) - 1.0
                for blk in range(NBLK):
                    V = PI * D1[nr, j] - 5.0 * PI / 4.0 - PI * K[blk]
                    Vh = V.astype(_md.bfloat16).astype(np.float64)
                    lhs_z[ln, base + 9 + blk] = Vh
                    lhs_z[ln, base + 9 + NBLK + blk] = V - Vh
                # each row shifted by -R: c1+c2+c3 == 1 -> vp = vm - R
                lhs_v[ln, seg * 3 + 0] = Up[nr, j] - Rc
                lhs_v[ln, seg * 3 + 1] = Uc[nr, j] - Rc
                lhs_v[ln, seg * 3 + 2] = Un[nr, j] - Rc
        nco = noise_h[core * NPC : (core + 1) * NPC]  # [NPC, T]
        nmat = np.zeros((P, CFD), np.float16)
        for ln in range(NPC):
            nmat[:, ln * FD : (ln + 1) * FD] = nco[ln].reshape(P, FD)
        zblob = np.zeros((KZ, FD + NPC * P), ml_dtypes.bfloat16)
        for ln in range(NPC):
            zblob[:, ln * P : (ln + 1) * P] = lhs_z[ln]
        zblob[:, NPC * P :] = rhs_z
        vblob = np.zeros((12, FD + NPC * P), np.float16)
        vblob[:, :FD] = rhs_v
        for ln in range(NPC):
            vblob[:, FD + ln * P : FD + (ln + 1) * P] = lhs_v[ln].astype(
                np.float16
            )
        in_maps.append({"noise": nmat, "zblob": zblob, "vblob": vblob})
    return in_maps, sc


def kernel(f0, uv, weight, noise):
    global LAST_RESULTS
    from concourse.bass_utils import run_bass_kernel_spmd

    in_maps, sc = _host_inputs(f0, uv, weight, noise)
    key = tuple(np.asarray(weight, np.float64).reshape(-1).tolist())
    if key not in _NC_CACHE:
        _NC_CACHE[key] = _build_nc(sc)
    nc = _NC_CACHE[key]
    res = run_bass_kernel_spmd(nc, in_maps, list(range(NCORES)))
    LAST_RESULTS = res
    out = np.empty((N, 1, T), dtype=np.float32)
    for core in range(NCORES):
        om = res.results[core]["out"].astype(np.float32)  # [P, CFD]
        for ln in range(NPC):
            out[core * NPC + ln, 0, :] = om[:, ln * FD : (ln + 1) * FD].reshape(T)
    return out
